# revision 23
# baseline (speedup 1.0000x reference)
"""Trainium2 Bass kernel for the BaselinePreprocessor problem.

Computes, for full inputs:
  fused = concat([interp(vision->T), interp(proprio->T), imu], -1)  # [64,1024,550]
  vox_mean = mean(occupancy grid 64^3 of 10k points)               # scalar
  out = concat([fused, vox_mean bcast], -1)                        # [64,1024,551]

Strategy (v7):
- Pure data parallel over batch (8 cores x 8 batches).
- Interp along time is a dense fp16 TensorE matmul with host-precomputed
  weights (tolerance 2e-2 >> fp16 error ~1e-3).
- Output rows are written in ROW-GROUPED layout: SBUF partition p holds
  output rows 512*blk + 4p + r (r=0..3), so each output DMA descriptor
  covers 4 consecutive full 551-wide rows = 8816 contiguous bytes.
  (2.2KB descriptors run at ~190GB/s aggregate; ~8.8KB run near peak.)
  The weight matrices are column-permuted ON THE HOST so the matmuls
  produce this layout directly; imu is host-reshaped to match. The vox
  column is DMA'd as garbage in the stream and patched at the end.
- Voxel occupancy: each core scatters its 1250-point shard into FIVE
  separate DRAM grids (two indirect-DMA columns each) so consecutive
  scatters don't serialize on write-after-write completion waits,
  unions them with 4 DVE adds, AllReduce(max)es the 512KB union across
  cores (the ~30-50us mesh latency overlaps the output stream), then a
  Sign activation with accum_out counts occupied voxels locally.
"""

import numpy as np

import concourse.bacc as bacc
import concourse.bass as bass
import concourse.mybir as mybir
import concourse.tile as tile
from concourse.bass_utils import run_bass_kernel_spmd

F32 = mybir.dt.float32
F16 = mybir.dt.float16
BF16 = mybir.dt.bfloat16
FP8 = mybir.dt.float8e4
I32 = mybir.dt.int32
ALU = mybir.AluOpType
ACT = mybir.ActivationFunctionType

N_CORES = 8
B = 8                      # batches per core
T = 1024
LV, CV = 64, 512           # vision input time-len, channels
LP, CP = 256, 32           # proprio
CI = 6                     # imu channels (identity interp: L == T)
C_OUT = 551
GRID = 64
NVOX = GRID * GRID * GRID  # 262144
NPTS = 10000
NPTS_CORE = NPTS // N_CORES           # 1250 points scattered per core
PTS_P, PTS_F = 125, NPTS_CORE // 125  # [125, 10] per-core point layout
N_GRIDS = 3                # scatter grids (3-4 idx columns each, no WAW stall)
N_BLK = 1                  # single row-block of 1024 output rows
R = 8                      # rows per partition (descriptor = R*2204 bytes)


def _interp_weights_T(L: int) -> np.ndarray:
    """W^T [L, T] with W the [T, L] linear-interp matrix (align_corners)."""
    scale = np.float32((L - 1) / (T - 1))
    pos = np.arange(T, dtype=np.float32) * scale
    lo = np.clip(np.floor(pos).astype(np.int32), 0, L - 1)
    hi = np.minimum(lo + 1, L - 1)
    w = (pos - lo.astype(np.float32)).astype(np.float32)
    wt = np.zeros((L, T), dtype=np.float32)
    np.add.at(wt, (lo, np.arange(T)), np.float32(1.0) - w)
    np.add.at(wt, (hi, np.arange(T)), w)
    return np.ascontiguousarray(wt)


def _rowgroup_perm() -> np.ndarray:
    """Column permutation: slot (blk, r, p) <- output row 1024*blk + R*p + r."""
    t = np.arange(T).reshape(N_BLK, 128, R)      # [blk, p, r] = 1024blk+Rp+r
    return t.transpose(0, 2, 1).reshape(-1)      # [blk, r, p] order


def _wp_chunks_needed(blk: int, r: int) -> list[int]:
    """Which K=128 row chunks of W_p^T have nonzeros for slot (blk, r)."""
    t = (T // N_BLK) * blk + R * np.arange(128) + r
    pos = t.astype(np.float32) * np.float32((LP - 1) / (T - 1))
    lo = np.clip(np.floor(pos).astype(np.int32), 0, LP - 1)
    hi = np.minimum(lo + 1, LP - 1)
    return sorted(set(lo // 128) | set(hi // 128))


def _emit(nc: bass.Bass, tc: tile.TileContext, ctx):
    vis = nc.declare_dram_parameter("vis", [LV, B, CV], F16, isOutput=False)
    pro = nc.declare_dram_parameter("pro", [LP, B, CP], F16, isOutput=False)
    # imu host-reshaped to [blk, p, r, b, c]
    imu = nc.declare_dram_parameter("imu", [N_BLK, 128, R, B, CI], F32, isOutput=False)
    points = nc.declare_dram_parameter("points", [NPTS_CORE, 3], F32, isOutput=False)
    wv = nc.declare_dram_parameter("wv", [LV, T], F16, isOutput=False)
    wp = nc.declare_dram_parameter("wp", [LP, T], F16, isOutput=False)
    out = nc.declare_dram_parameter("out", [B, T, C_OUT], F32, isOutput=True)

    grids = [nc.dram_tensor(f"grid{g}", [NVOX, 1], FP8) for g in range(N_GRIDS)]
    grid_u = nc.dram_tensor("grid_u", [NVOX, 1], FP8)
    grid_sh = nc.dram_tensor("grid_sh", [NVOX, 1], FP8, addr_space="Shared")

    def g2d(t):
        return t[:].rearrange("(p f) o -> p (f o)", p=128)  # [128, 2048]

    const = ctx.enter_context(tc.tile_pool(name="const", bufs=1))
    vxw = ctx.enter_context(tc.tile_pool(name="vxw", bufs=1))
    outp = ctx.enter_context(tc.tile_pool(name="outp", bufs=3))
    psumv = ctx.enter_context(tc.tile_pool(name="psumv", bufs=4, space="PSUM"))
    psump = ctx.enter_context(tc.tile_pool(name="psump", bufs=3, space="PSUM"))
    psums = ctx.enter_context(tc.tile_pool(name="psums", bufs=1, space="PSUM"))

    # ---------------- loads + grid zeroing ----------------
    pts_sb = vxw.tile([PTS_P, PTS_F, 3], F32)
    nc.sync.dma_start(out=pts_sb[:], in_=points[:].rearrange("(p f) c -> p f c", p=PTS_P))
    zer = const.tile([128, NVOX // 128], FP8)
    nc.gpsimd.memset(zer[:], 0.0)
    for g in range(N_GRIDS):
        eng = nc.sync if g % 2 == 0 else nc.scalar
        eng.dma_start(out=g2d(grids[g]), in_=zer[:])
    wv_sb = const.tile([LV, T], F16)
    nc.scalar.dma_start(out=wv_sb[:], in_=wv[:])
    wp_sb = const.tile([128, 2, T], F16)
    nc.scalar.dma_start(out=wp_sb[:], in_=wp[:].rearrange("(k p) t -> p k t", p=128))
    vh_sb = const.tile([LV, B, CV], F16)
    nc.scalar.dma_start(out=vh_sb[:], in_=vis[:])
    pro_sb = const.tile([128, 2, B, CP], F16)
    nc.scalar.dma_start(out=pro_sb[:], in_=pro[:].rearrange("(k p) b c -> p k b c", p=128))
    imu_sb = const.tile([128, N_BLK, R, B, CI], F32)
    nc.scalar.dma_start(out=imu_sb[:], in_=imu[:].rearrange("j p r b c -> p j r b c"))
    ones_col = const.tile([128, 1], F32)
    nc.gpsimd.memset(ones_col[:], 1.0)
    ones_row = const.tile([1, 128], F32)
    nc.gpsimd.memset(ones_row[:], 1.0)
    ones_pts = const.tile([PTS_P, 1], FP8)
    nc.gpsimd.memset(ones_pts[:], 1.0)

    # ---------------- voxel index math (vector, leads the queue) ----------------
    # q_c = clip(trunc((p_c + 2) * 16), 0, 63), computed clip-then-floor
    # (equivalent: trunc==floor on the surviving non-negative range).
    # floor via int32 round-trip (any rounding mode) + is_gt correction.
    q = []
    ji = vxw.tile([PTS_P, PTS_F], I32)
    gt = vxw.tile([PTS_P, PTS_F], F32)
    for c in range(3):
        qc = vxw.tile([PTS_P, PTS_F], F32, tag=f"q{c}")
        nc.vector.tensor_scalar(qc[:], pts_sb[:, :, c], 2.0, 16.0, ALU.add, ALU.mult)
        nc.vector.tensor_scalar(qc[:], qc[:], 63.0, 0.0, ALU.min, ALU.max)
        rt = vxw.tile([PTS_P, PTS_F], F32, tag=f"rt{c}")
        nc.vector.tensor_copy(out=ji[:], in_=qc[:])
        nc.vector.tensor_copy(out=rt[:], in_=ji[:])
        nc.vector.tensor_tensor(gt[:], rt[:], qc[:], ALU.is_gt)
        nc.vector.tensor_tensor(qc[:], rt[:], gt[:], ALU.subtract)
        q.append(qc)
    acc = vxw.tile([PTS_P, PTS_F], F32)
    nc.vector.tensor_scalar(acc[:], q[0][:], 64.0, None, ALU.mult)
    nc.vector.tensor_tensor(acc[:], acc[:], q[1][:], ALU.add)
    nc.vector.tensor_scalar(acc[:], acc[:], 64.0, None, ALU.mult)
    nc.vector.tensor_tensor(acc[:], acc[:], q[2][:], ALU.add)
    idx = vxw.tile([PTS_P, PTS_F], I32)
    nc.vector.tensor_copy(out=idx[:], in_=acc[:])  # exact integers -> exact

    # scatter ones: one indirect DMA per idx column (<=128 offsets each);
    # columns f and f+5 share a grid, spaced 5 issues apart so the WAW
    # completion wait never stalls the queue
    for f in range(PTS_F):
        nc.gpsimd.indirect_dma_start(
            out=grids[f % N_GRIDS][:],
            out_offset=bass.IndirectOffsetOnAxis(ap=idx[:, f:f + 1], axis=0),
            in_=ones_pts[:],
            in_offset=None,
        )

    # ---------------- proprio prepass pieces ----------------
    pp_tiles = [
        const.tile([128, R, B, CP], F32, tag=f"ppsb{j}", name=f"ppsb{j}")
        for j in range(N_BLK)
    ]

    def emit_pp(blk):
        for r in range(R):
            ppj = psump.tile([128, B, CP], F32, tag="pp")
            cs = slice(128 * (R * blk + r), 128 * (R * blk + r + 1))
            ks = _wp_chunks_needed(blk, r)
            for i, k in enumerate(ks):
                nc.tensor.matmul(
                    out=ppj[:],
                    lhsT=wp_sb[:, k, cs],
                    rhs=pro_sb[:, k, :, :],
                    start=(i == 0),
                    stop=(i == len(ks) - 1),
                )
            # ACT copies PSUM->SBUF; keeps DVE free for the stream
            nc.scalar.activation(out=pp_tiles[blk][:, r, :, :], in_=ppj[:], func=ACT.Copy)

    # readback tiles for the grid union
    rb = [
        vxw.tile([128, NVOX // 128], FP8, tag=f"rb{g}", name=f"rb{g}")
        for g in range(N_GRIDS)
    ]

    def emit_grid_readback():
        # on gpsimd: its queue is idle after the scatters, and a readback
        # waiting on a scatter's completion must not stall the stream queues.
        # accum_op folds the union into the readback chain (no DVE involved)
        for g in range(N_GRIDS):
            nc.gpsimd.dma_start(
                out=rb[0][:], in_=g2d(grids[g]),
                accum_op=ALU.bypass if g == 0 else ALU.add,
            )

    def emit_union():
        # counts 0..10 are exact in fp8e4m3 (ints <= 16); sum>0 iff occupied
        for g in range(1, N_GRIDS):
            nc.vector.tensor_tensor(rb[0][:], rb[0][:], rb[g][:], ALU.add)

    # ---------------- main stream ----------------
    # per (blk, b): 4 row-slot matmuls -> [128, 4, 551] tile -> ONE DMA of
    # 8816B-per-partition contiguous descriptors (col 550 garbage, patched)
    for blk in range(N_BLK):
        emit_pp(blk)
        for b in range(B):
            ob = outp.tile([128, R, C_OUT], F32, tag="ob")
            for r in range(R):
                cs = slice(128 * (R * blk + r), 128 * (R * blk + r + 1))
                pv = psumv.tile([128, CV], F32, tag="pv")
                nc.tensor.matmul(
                    out=pv[:], lhsT=wv_sb[:, cs], rhs=vh_sb[:, b, :],
                    start=True, stop=True,
                )
                # split the PSUM->SBUF copies between DVE and ACT
                if (b + r) % 2 == 0:
                    nc.vector.tensor_copy(out=ob[:, r, 0:CV], in_=pv[:])
                else:
                    nc.scalar.activation(out=ob[:, r, 0:CV], in_=pv[:], func=ACT.Copy)
            nc.vector.tensor_copy(out=ob[:, :, CV:CV + CP], in_=pp_tiles[blk][:, :, b, :])
            nc.vector.tensor_copy(out=ob[:, :, 544:550], in_=imu_sb[:, blk, :, b, :])
            eng = nc.sync if b % 2 == 0 else nc.scalar
            eng.dma_start(
                out=out[b, (T // N_BLK) * blk:(T // N_BLK) * (blk + 1), :].rearrange(
                    "(p r) c -> p r c", p=128),
                in_=ob[:],
            )
            if blk == 0 and b == 0:
                emit_grid_readback()

    # ---------------- union -> AllReduce(max) -> count -> vox patches ----------------
    nc.gpsimd.dma_start(out=g2d(grid_u), in_=rb[0][:])
    nc.gpsimd.collective_compute(
        "AllReduce",
        ALU.max,
        replica_groups=[list(range(N_CORES))],
        ins=[grid_u[:]],
        outs=[grid_sh[:]],
    )
    rbu = vxw.tile([128, NVOX // 128], FP8)
    nc.gpsimd.dma_start(out=rbu[:], in_=g2d(grid_sh))
    occ = vxw.tile([128, NVOX // 128], FP8)
    red = vxw.tile([128, 1], F32)
    # Sign(count): counts >= 0 -> exactly the 0/1 occupancy; accum_out row-sums
    nc.scalar.activation(out=occ[:], in_=rbu[:], func=ACT.Sign, accum_out=red[:])
    cnt_ps = psums.tile([1, 1], F32, tag="cnt")
    nc.tensor.matmul(out=cnt_ps[:], lhsT=red[:], rhs=ones_col[:], start=True, stop=True)
    cnt_sb = vxw.tile([1, 1], F32)
    nc.scalar.activation(out=cnt_sb[:], in_=cnt_ps[:], func=ACT.Copy)
    vox1 = vxw.tile([1, 1], F32)
    nc.gpsimd.tensor_scalar(vox1[:], cnt_sb[:], 1.0 / NVOX, None, ALU.mult)
    vox_pb = psumv.tile([128, CV], F32, tag="pv")
    nc.tensor.matmul(
        out=vox_pb[:, 0:1], lhsT=ones_row[:], rhs=vox1[:], start=True, stop=True
    )
    vox_col = vxw.tile([128, T // 128], F32)
    nc.scalar.activation(
        out=vox_col[:], in_=vox_pb[:, 0:1].to_broadcast([128, T // 128]), func=ACT.Copy
    )
    for b in range(B):
        eng = nc.sync if b % 2 == 0 else nc.scalar
        eng.dma_start(
            out=out[b, :, 550:551].rearrange("(j p) o -> p (j o)", p=128),
            in_=vox_col[:],
        )


_CACHE: dict[str, object] = {}


def _get_nc() -> bass.Bass:
    if "nc" not in _CACHE:
        from contextlib import ExitStack

        # Bacc (not plain Bass): its finalize() legalizes sync waits (HW
        # allows at most one wait per instruction; extras are split into
        # event-semaphore instructions).
        nc = bacc.Bacc(None, num_devices=N_CORES)
        with ExitStack() as ctx:
            tc = ctx.enter_context(tile.TileContext(nc))
            _emit(nc, tc, ctx)
        if not nc.is_finalized():
            nc.finalize()
        _CACHE["nc"] = nc
    return _CACHE["nc"]  # type: ignore[return-value]


def _run(inputs: dict, trace: bool = False):
    vision = np.asarray(inputs["vision"], dtype=np.float32)
    proprio = np.asarray(inputs["proprio"], dtype=np.float32)
    imu = np.asarray(inputs["imu"], dtype=np.float32)
    points = np.ascontiguousarray(np.asarray(inputs["points"], dtype=np.float32))
    perm = _rowgroup_perm()
    wv16 = _interp_weights_T(LV)[:, perm].astype(np.float16)
    wp16 = _interp_weights_T(LP)[:, perm].astype(np.float16)
    wv16 = np.ascontiguousarray(wv16)
    wp16 = np.ascontiguousarray(wp16)

    nc = _get_nc()
    in_maps = []
    for i in range(N_CORES):
        sl = slice(i * B, (i + 1) * B)
        psl = slice(i * NPTS_CORE, (i + 1) * NPTS_CORE)
        in_maps.append({
            "vis": np.ascontiguousarray(
                vision[sl].transpose(1, 0, 2).astype(np.float16)),
            "pro": np.ascontiguousarray(
                proprio[sl].transpose(1, 0, 2).astype(np.float16)),
            # [T,B,C] -> [blk, p, r, b, c] with T = 512*blk + 4p + r
            "imu": np.ascontiguousarray(
                imu[sl].transpose(1, 0, 2).reshape(N_BLK, 128, R, B, CI)),
            "points": np.ascontiguousarray(points[psl]),
            "wv": wv16,
            "wp": wp16,
        })
    res = run_bass_kernel_spmd(nc, in_maps, list(range(N_CORES)), trace=trace)
    full = np.concatenate([res.results[i]["out"] for i in range(N_CORES)], axis=0)
    return full, res


def kernel(**inputs) -> np.ndarray:
    full, _ = _run(inputs)
    return full


# revision 24
# speedup vs baseline: 1.2195x; 1.2195x over previous
"""Trainium2 Bass kernel for the BaselinePreprocessor problem.

Computes, for full inputs:
  fused = concat([interp(vision->T), interp(proprio->T), imu], -1)  # [64,1024,550]
  vox_mean = mean(occupancy grid 64^3 of 10k points)               # scalar
  out = concat([fused, vox_mean bcast], -1)                        # [64,1024,551]

Strategy (v7):
- Pure data parallel over batch (8 cores x 8 batches).
- Interp along time is a dense fp16 TensorE matmul with host-precomputed
  weights (tolerance 2e-2 >> fp16 error ~1e-3).
- Output rows are written in ROW-GROUPED layout: SBUF partition p holds
  output rows 512*blk + 4p + r (r=0..3), so each output DMA descriptor
  covers 4 consecutive full 551-wide rows = 8816 contiguous bytes.
  (2.2KB descriptors run at ~190GB/s aggregate; ~8.8KB run near peak.)
  The weight matrices are column-permuted ON THE HOST so the matmuls
  produce this layout directly; imu is host-reshaped to match. The vox
  column is DMA'd as garbage in the stream and patched at the end.
- Voxel occupancy: each core scatters its 1250-point shard into FIVE
  separate DRAM grids (two indirect-DMA columns each) so consecutive
  scatters don't serialize on write-after-write completion waits,
  unions them with 4 DVE adds, AllReduce(max)es the 512KB union across
  cores (the ~30-50us mesh latency overlaps the output stream), then a
  Sign activation with accum_out counts occupied voxels locally.
"""

import numpy as np

import concourse.bacc as bacc
import concourse.bass as bass
import concourse.mybir as mybir
import concourse.tile as tile
from concourse.bass_utils import run_bass_kernel_spmd

F32 = mybir.dt.float32
F16 = mybir.dt.float16
BF16 = mybir.dt.bfloat16
FP8 = mybir.dt.float8e4
I32 = mybir.dt.int32
ALU = mybir.AluOpType
ACT = mybir.ActivationFunctionType

N_CORES = 8
B = 8                      # batches per core
T = 1024
LV, CV = 64, 512           # vision input time-len, channels
LP, CP = 256, 32           # proprio
CI = 6                     # imu channels (identity interp: L == T)
C_OUT = 551
GRID = 64
NVOX = GRID * GRID * GRID  # 262144
NPTS = 10000
NPTS_CORE = NPTS // N_CORES           # 1250 points scattered per core
PTS_P, PTS_F = 125, NPTS_CORE // 125  # [125, 10] per-core point layout
N_GRIDS = 3                # scatter grids (3-4 idx columns each, no WAW stall)
N_BLK = 1                  # single row-block of 1024 output rows
R = 8                      # rows per partition (descriptor = R*2204 bytes)


def _interp_weights_T(L: int) -> np.ndarray:
    """W^T [L, T] with W the [T, L] linear-interp matrix (align_corners)."""
    scale = np.float32((L - 1) / (T - 1))
    pos = np.arange(T, dtype=np.float32) * scale
    lo = np.clip(np.floor(pos).astype(np.int32), 0, L - 1)
    hi = np.minimum(lo + 1, L - 1)
    w = (pos - lo.astype(np.float32)).astype(np.float32)
    wt = np.zeros((L, T), dtype=np.float32)
    np.add.at(wt, (lo, np.arange(T)), np.float32(1.0) - w)
    np.add.at(wt, (hi, np.arange(T)), w)
    return np.ascontiguousarray(wt)


def _rowgroup_perm() -> np.ndarray:
    """Column permutation: slot (blk, r, p) <- output row 1024*blk + R*p + r."""
    t = np.arange(T).reshape(N_BLK, 128, R)      # [blk, p, r] = 1024blk+Rp+r
    return t.transpose(0, 2, 1).reshape(-1)      # [blk, r, p] order


def _wp_chunks_needed(blk: int, r: int) -> list[int]:
    """Which K=128 row chunks of W_p^T have nonzeros for slot (blk, r)."""
    t = (T // N_BLK) * blk + R * np.arange(128) + r
    pos = t.astype(np.float32) * np.float32((LP - 1) / (T - 1))
    lo = np.clip(np.floor(pos).astype(np.int32), 0, LP - 1)
    hi = np.minimum(lo + 1, LP - 1)
    return sorted(set(lo // 128) | set(hi // 128))


def _emit(nc: bass.Bass, tc: tile.TileContext, ctx):
    vis = nc.declare_dram_parameter("vis", [LV, B, CV], F16, isOutput=False)
    pro = nc.declare_dram_parameter("pro", [LP, B, CP], F16, isOutput=False)
    # imu host-reshaped to [blk, p, r, b, c]
    imu = nc.declare_dram_parameter("imu", [N_BLK, 128, R, B, CI], F32, isOutput=False)
    points = nc.declare_dram_parameter("points", [NPTS_CORE, 3], F32, isOutput=False)
    wv = nc.declare_dram_parameter("wv", [LV, T], F16, isOutput=False)
    wp = nc.declare_dram_parameter("wp", [LP, T], F16, isOutput=False)
    out = nc.declare_dram_parameter("out", [B, T, C_OUT], F32, isOutput=True)

    grids = [nc.dram_tensor(f"grid{g}", [NVOX, 1], FP8) for g in range(N_GRIDS)]
    grid_u = nc.dram_tensor("grid_u", [NVOX, 1], FP8)
    grid_sh = nc.dram_tensor("grid_sh", [NVOX, 1], FP8, addr_space="Shared")

    def g2d(t):
        return t[:].rearrange("(p f) o -> p (f o)", p=128)  # [128, 2048]

    const = ctx.enter_context(tc.tile_pool(name="const", bufs=1))
    vxw = ctx.enter_context(tc.tile_pool(name="vxw", bufs=1))
    outp = ctx.enter_context(tc.tile_pool(name="outp", bufs=3))
    psumv = ctx.enter_context(tc.tile_pool(name="psumv", bufs=4, space="PSUM"))
    psump = ctx.enter_context(tc.tile_pool(name="psump", bufs=3, space="PSUM"))
    psums = ctx.enter_context(tc.tile_pool(name="psums", bufs=1, space="PSUM"))

    # ---------------- loads + grid zeroing ----------------
    pts_sb = vxw.tile([PTS_P, PTS_F, 3], F32)
    nc.sync.dma_start(out=pts_sb[:], in_=points[:].rearrange("(p f) c -> p f c", p=PTS_P))
    zer = const.tile([128, NVOX // 128], FP8)
    nc.gpsimd.memset(zer[:], 0.0)
    for g in range(N_GRIDS):
        eng = nc.sync if g % 2 == 0 else nc.scalar
        eng.dma_start(out=g2d(grids[g]), in_=zer[:])
    wv_sb = const.tile([LV, T], F16)
    nc.scalar.dma_start(out=wv_sb[:], in_=wv[:])
    wp_sb = const.tile([128, 2, T], F16)
    nc.scalar.dma_start(out=wp_sb[:], in_=wp[:].rearrange("(k p) t -> p k t", p=128))
    vh_sb = const.tile([LV, B, CV], F16)
    nc.scalar.dma_start(out=vh_sb[:], in_=vis[:])
    pro_sb = const.tile([128, 2, B, CP], F16)
    nc.scalar.dma_start(out=pro_sb[:], in_=pro[:].rearrange("(k p) b c -> p k b c", p=128))
    imu_sb = const.tile([128, N_BLK, R, B, CI], F32)
    nc.scalar.dma_start(out=imu_sb[:], in_=imu[:].rearrange("j p r b c -> p j r b c"))
    ones_col = const.tile([128, 1], F32)
    nc.gpsimd.memset(ones_col[:], 1.0)
    ones_row = const.tile([1, 128], F32)
    nc.gpsimd.memset(ones_row[:], 1.0)
    ones_pts = const.tile([PTS_P, 1], FP8)
    nc.gpsimd.memset(ones_pts[:], 1.0)

    # ---------------- voxel index math (vector, leads the queue) ----------------
    # q_c = clip(trunc((p_c + 2) * 16), 0, 63), computed clip-then-floor
    # (equivalent: trunc==floor on the surviving non-negative range).
    # floor via int32 round-trip (any rounding mode) + is_gt correction.
    q = []
    ji = vxw.tile([PTS_P, PTS_F], I32)
    gt = vxw.tile([PTS_P, PTS_F], F32)
    for c in range(3):
        qc = vxw.tile([PTS_P, PTS_F], F32, tag=f"q{c}")
        nc.vector.tensor_scalar(qc[:], pts_sb[:, :, c], 2.0, 16.0, ALU.add, ALU.mult)
        nc.vector.tensor_scalar(qc[:], qc[:], 63.0, 0.0, ALU.min, ALU.max)
        rt = vxw.tile([PTS_P, PTS_F], F32, tag=f"rt{c}")
        nc.vector.tensor_copy(out=ji[:], in_=qc[:])
        nc.vector.tensor_copy(out=rt[:], in_=ji[:])
        nc.vector.tensor_tensor(gt[:], rt[:], qc[:], ALU.is_gt)
        nc.vector.tensor_tensor(qc[:], rt[:], gt[:], ALU.subtract)
        q.append(qc)
    acc = vxw.tile([PTS_P, PTS_F], F32)
    nc.vector.tensor_scalar(acc[:], q[0][:], 64.0, None, ALU.mult)
    nc.vector.tensor_tensor(acc[:], acc[:], q[1][:], ALU.add)
    nc.vector.tensor_scalar(acc[:], acc[:], 64.0, None, ALU.mult)
    nc.vector.tensor_tensor(acc[:], acc[:], q[2][:], ALU.add)
    idx = vxw.tile([PTS_P, PTS_F], I32)
    nc.vector.tensor_copy(out=idx[:], in_=acc[:])  # exact integers -> exact

    # scatter ones: one indirect DMA per idx column (<=128 offsets each);
    # columns f and f+5 share a grid, spaced 5 issues apart so the WAW
    # completion wait never stalls the queue
    for f in range(PTS_F):
        nc.gpsimd.indirect_dma_start(
            out=grids[f % N_GRIDS][:],
            out_offset=bass.IndirectOffsetOnAxis(ap=idx[:, f:f + 1], axis=0),
            in_=ones_pts[:],
            in_offset=None,
        )

    # ---------------- proprio prepass pieces ----------------
    pp_tiles = [
        const.tile([128, R, B, CP], F32, tag=f"ppsb{j}", name=f"ppsb{j}")
        for j in range(N_BLK)
    ]

    def emit_pp(blk):
        for r in range(R):
            ppj = psump.tile([128, B, CP], F32, tag="pp")
            cs = slice(128 * (R * blk + r), 128 * (R * blk + r + 1))
            ks = _wp_chunks_needed(blk, r)
            for i, k in enumerate(ks):
                nc.tensor.matmul(
                    out=ppj[:],
                    lhsT=wp_sb[:, k, cs],
                    rhs=pro_sb[:, k, :, :],
                    start=(i == 0),
                    stop=(i == len(ks) - 1),
                )
            # ACT copies PSUM->SBUF; keeps DVE free for the stream
            nc.scalar.activation(out=pp_tiles[blk][:, r, :, :], in_=ppj[:], func=ACT.Copy)

    # readback tiles for the grid union
    rb = [
        vxw.tile([128, NVOX // 128], FP8, tag=f"rb{g}", name=f"rb{g}")
        for g in range(N_GRIDS)
    ]

    def emit_grid_readback():
        # on gpsimd: its queue is idle after the scatters, and a readback
        # waiting on a scatter's completion must not stall the stream queues.
        # independent tiles -> the three readbacks stay in flight together
        for g in range(N_GRIDS):
            nc.gpsimd.dma_start(out=rb[g][:], in_=g2d(grids[g]))

    def emit_union():
        # counts 0..10 are exact in fp8e4m3 (ints <= 16); sum>0 iff occupied
        for g in range(1, N_GRIDS):
            nc.vector.tensor_tensor(rb[0][:], rb[0][:], rb[g][:], ALU.add)

    # ---------------- main stream ----------------
    # per (blk, b): 4 row-slot matmuls -> [128, 4, 551] tile -> ONE DMA of
    # 8816B-per-partition contiguous descriptors (col 550 garbage, patched)
    for blk in range(N_BLK):
        emit_pp(blk)
        for b in range(B):
            ob = outp.tile([128, R, C_OUT], F32, tag="ob")
            for r in range(R):
                cs = slice(128 * (R * blk + r), 128 * (R * blk + r + 1))
                pv = psumv.tile([128, CV], F32, tag="pv")
                nc.tensor.matmul(
                    out=pv[:], lhsT=wv_sb[:, cs], rhs=vh_sb[:, b, :],
                    start=True, stop=True,
                )
                # split the PSUM->SBUF copies between DVE and ACT
                if (b + r) % 2 == 0:
                    nc.vector.tensor_copy(out=ob[:, r, 0:CV], in_=pv[:])
                else:
                    nc.scalar.activation(out=ob[:, r, 0:CV], in_=pv[:], func=ACT.Copy)
            nc.vector.tensor_copy(out=ob[:, :, CV:CV + CP], in_=pp_tiles[blk][:, :, b, :])
            nc.vector.tensor_copy(out=ob[:, :, 544:550], in_=imu_sb[:, blk, :, b, :])
            eng = nc.sync if b % 2 == 0 else nc.scalar
            eng.dma_start(
                out=out[b, (T // N_BLK) * blk:(T // N_BLK) * (blk + 1), :].rearrange(
                    "(p r) c -> p r c", p=128),
                in_=ob[:],
            )
            if blk == 0 and b == 0:
                emit_grid_readback()
            if blk == 0 and b == 5:
                emit_union()

    # ---------------- union -> AllReduce(max) -> count -> vox patches ----------------
    nc.gpsimd.dma_start(out=g2d(grid_u), in_=rb[0][:])
    nc.gpsimd.collective_compute(
        "AllReduce",
        ALU.max,
        replica_groups=[list(range(N_CORES))],
        ins=[grid_u[:]],
        outs=[grid_sh[:]],
    )
    rbu = vxw.tile([128, NVOX // 128], FP8)
    nc.gpsimd.dma_start(out=rbu[:], in_=g2d(grid_sh))
    occ = vxw.tile([128, NVOX // 128], FP8)
    red = vxw.tile([128, 1], F32)
    # Sign(count): counts >= 0 -> exactly the 0/1 occupancy; accum_out row-sums
    nc.scalar.activation(out=occ[:], in_=rbu[:], func=ACT.Sign, accum_out=red[:])
    cnt_ps = psums.tile([1, 1], F32, tag="cnt")
    nc.tensor.matmul(out=cnt_ps[:], lhsT=red[:], rhs=ones_col[:], start=True, stop=True)
    cnt_sb = vxw.tile([1, 1], F32)
    nc.scalar.activation(out=cnt_sb[:], in_=cnt_ps[:], func=ACT.Copy)
    vox1 = vxw.tile([1, 1], F32)
    nc.gpsimd.tensor_scalar(vox1[:], cnt_sb[:], 1.0 / NVOX, None, ALU.mult)
    vox_pb = psumv.tile([128, CV], F32, tag="pv")
    nc.tensor.matmul(
        out=vox_pb[:, 0:1], lhsT=ones_row[:], rhs=vox1[:], start=True, stop=True
    )
    vox_col = vxw.tile([128, T // 128], F32)
    nc.scalar.activation(
        out=vox_col[:], in_=vox_pb[:, 0:1].to_broadcast([128, T // 128]), func=ACT.Copy
    )
    for b in range(B):
        eng = nc.sync if b % 2 == 0 else nc.scalar
        eng.dma_start(
            out=out[b, :, 550:551].rearrange("(j p) o -> p (j o)", p=128),
            in_=vox_col[:],
        )


_CACHE: dict[str, object] = {}


def _get_nc() -> bass.Bass:
    if "nc" not in _CACHE:
        from contextlib import ExitStack

        # Bacc (not plain Bass): its finalize() legalizes sync waits (HW
        # allows at most one wait per instruction; extras are split into
        # event-semaphore instructions).
        nc = bacc.Bacc(None, num_devices=N_CORES)
        with ExitStack() as ctx:
            tc = ctx.enter_context(tile.TileContext(nc))
            _emit(nc, tc, ctx)
        if not nc.is_finalized():
            nc.finalize()
        _CACHE["nc"] = nc
    return _CACHE["nc"]  # type: ignore[return-value]


def _run(inputs: dict, trace: bool = False):
    vision = np.asarray(inputs["vision"], dtype=np.float32)
    proprio = np.asarray(inputs["proprio"], dtype=np.float32)
    imu = np.asarray(inputs["imu"], dtype=np.float32)
    points = np.ascontiguousarray(np.asarray(inputs["points"], dtype=np.float32))
    perm = _rowgroup_perm()
    wv16 = _interp_weights_T(LV)[:, perm].astype(np.float16)
    wp16 = _interp_weights_T(LP)[:, perm].astype(np.float16)
    wv16 = np.ascontiguousarray(wv16)
    wp16 = np.ascontiguousarray(wp16)

    nc = _get_nc()
    in_maps = []
    for i in range(N_CORES):
        sl = slice(i * B, (i + 1) * B)
        psl = slice(i * NPTS_CORE, (i + 1) * NPTS_CORE)
        in_maps.append({
            "vis": np.ascontiguousarray(
                vision[sl].transpose(1, 0, 2).astype(np.float16)),
            "pro": np.ascontiguousarray(
                proprio[sl].transpose(1, 0, 2).astype(np.float16)),
            # [T,B,C] -> [blk, p, r, b, c] with T = 512*blk + 4p + r
            "imu": np.ascontiguousarray(
                imu[sl].transpose(1, 0, 2).reshape(N_BLK, 128, R, B, CI)),
            "points": np.ascontiguousarray(points[psl]),
            "wv": wv16,
            "wp": wp16,
        })
    res = run_bass_kernel_spmd(nc, in_maps, list(range(N_CORES)), trace=trace)
    full = np.concatenate([res.results[i]["out"] for i in range(N_CORES)], axis=0)
    return full, res


def kernel(**inputs) -> np.ndarray:
    full, _ = _run(inputs)
    return full
